# revision 53
# baseline (speedup 1.0000x reference)
"""Trainium2 Bass kernel for ExponentialConcordanceLoss.

Reference semantics (N = 8192):
    t = targets[:, 0]; e = targets[:, 1] != 0; s = preds
    mask[j, i] = (t[i] < t[j]) & e[i]            (all inputs finite)
    loss = sum_{j,i} mask * exp(s[j] - s[i]) / max(sum(mask), 1)

Factorization used on device:
    loss_sum = sum_j exp(s[j]) * (sum_i mask[j,i] * exp(-s[i]))
    count    = sum_{j,i} mask[j,i]

v5 layout: both axes are sorted by time on the host (pure index/layout
prep). The i-axis keeps only event rows, in blocks of 128 sorted by
time; the j-axis is the full 8192 sorted list, in 64 chunks of 128.
For an i-block with time range [tmin, tmax], columns left of
searchsorted(tmin) give mask 0 and columns right of searchsorted(tmax)
give mask 1 for every (real-event) row.  Only the narrow "band" between
the two - about 2-4 chunks of 128 - needs elementwise comparisons; the
all-ones far region collapses to
    loss_far  = (sum_i exp(-s_i) over block) * (sum_j exp(s_j) over far)
    count_far = n_events_in_block * n_far_columns
with the far-side sums produced on device by ones-vector matmuls and a
host 0/1 chunk-selector matrix.

Inside a band window the comparison (t_i < t_j) is rewritten as
(rank_window(j) >= R_i) where R_i = searchsorted(t_sorted, t_i, 'right')
relativized to the window - exact for ties, computed on host as layout
metadata, and compared on device against an int16 iota.  This removes
the [128 x 8192] t-broadcast entirely: band compares run on DVE in 4x
mode, mask chunks feed 1-column bf16 matmuls accumulating per-j psum U,
and a single fused epilogue op reduces (U | far_coeffs) * exp(s_j).

Blocks are dealt into "slots" of 8 (one block per core per slot) by
similar band width, so the compiled program (shared by all cores) has
one static band width per slot and the cores stay balanced.  All
per-core tensors ride in ONE merged input DMA; the output goes out
through a SWDGE scatter-add descriptor prepared early on the idle
GPSIMD queue and triggered when the epilogue lands (the output region
is zero-filled by a second idle-time DMA first), skipping the long
HWDGE/DGE latency chain on the critical tail.
"""

import sys

if "/opt/trn_rl_repo" not in sys.path:
    sys.path.insert(0, "/opt/trn_rl_repo")

import numpy as np

N = 8192
NCORES = 8
NCH = N // 128         # j chunks of 128 (64)
USE_SCATTER_OUT = True

_CACHE = {}


def _build(widths):
    """Trace the SPMD Bass program for the given per-slot band widths
    (in 128-column chunks)."""
    import concourse.bass as bass
    import concourse.mybir as mybir

    if USE_SCATTER_OUT:
        # Bacc's compile pass legalizes the GPSIMD library-load pseudo
        # instruction that the SWDGE scatter-add output path needs
        import concourse.bacc as bacc

    f32 = mybir.dt.float32
    f32r = mybir.dt.float32r
    bf16 = mybir.dt.bfloat16
    i16 = mybir.dt.int16
    Alu = mybir.AluOpType
    Act = mybir.ActivationFunctionType
    X = mybir.AxisListType.X

    ns = len(widths)
    offs = [0]
    for w in widths:
        offs.append(offs[-1] + w)
    WT = offs[-1]                      # total band chunks
    WMAX = max(widths)
    PIN = 3 * ns + 64 + WT + 64        # merged input columns
    O_H = 3 * ns                       # hmat offset
    O_SB = 3 * ns + 64                 # sband offset
    O_SJ = 3 * ns + 64 + WT            # sjb offset

    nc = bacc.Bacc(None) if USE_SCATTER_OUT else bass.Bass()

    pin_d = nc.dram_tensor("pin", [128, PIN], f32, kind="ExternalInput")
    out_d = nc.dram_tensor("out", [128, 64], f32, kind="ExternalOutput")

    from contextlib import ExitStack

    with ExitStack() as ctx:
        en = ctx.enter_context
        pin_s = en(nc.sbuf_tensor([128, PIN], f32))
        iot = en(nc.sbuf_tensor([128, 128 * WMAX], i16))
        idx16 = en(nc.sbuf_tensor([128, 8], i16))
        idxc = en(nc.sbuf_tensor([128, 8], i16))
        w_f32 = en(nc.sbuf_tensor([128, ns], f32))
        wb16 = en(nc.sbuf_tensor([128, ns], bf16))
        vext = en(nc.sbuf_tensor([128, WT + 64], f32))
        ones_b = en(nc.sbuf_tensor([128, 128], f32))
        wcol_s = en(nc.sbuf_tensor([ns, 1], f32))
        wbb = en(nc.sbuf_tensor([ns, 128], f32))
        mbufs = [
            en(nc.sbuf_tensor(f"mbuf{q}", [128, 128 * w], bf16))
            for q, w in enumerate(widths)
        ]
        cntT = en(nc.sbuf_tensor([128, ns], f32))
        junke = en(nc.sbuf_tensor([128, WT + 64], f32))
        junkc = en(nc.sbuf_tensor([128, 2 * ns], f32))
        red = en(nc.sbuf_tensor([128, 1, 16], f32))
        ptile = en(nc.psum_tensor([128, WT + 64], f32))
        pwcol = en(nc.psum_tensor([ns, 1], f32))
        dsem = en(nc.semaphore())    # merged input
        xsem = en(nc.semaphore())    # clamped scatter indices ready
        isem = en(nc.semaphore())    # pool setup: 1=idx iota, 2=iota, 3=ones, 4=red0
        psem = en(nc.semaphore())    # scatter prep issued
        asem = en(nc.semaphore())    # ACT: 1=w, 2=vjb, 3=vband
        wbsem = en(nc.semaphore())   # wb16 ready
        vv = en(nc.semaphore())      # DVE progress
        m1sem = en(nc.semaphore())   # PE: wcol matmul done
        m2sem = en(nc.semaphore())   # PE: far matmul done (band psum also final)
        osem = en(nc.semaphore())    # out DMA
        block = en(nc.Block())

        VV_PIECES = ns
        VV_DONE = VV_PIECES + 5      # wcol, wbb, cntT red, Kcol red, fused stt

        @block.sync
        def _(sync):
            sync.dma_start(pin_s[:], pin_d[:]).then_inc(dsem, 16)
            if not USE_SCATTER_OUT:
                sync.wait_ge(vv, VV_DONE)
                sync.dma_start(out_d[:, 0:4], red[:, 0, 0:4]).then_inc(osem, 16)

        @block.scalar
        def _(scalar):
            # (Bacc's insert_act_table_loads pre-loads the Exp table before
            # the first activation, which carries the dsem wait itself)
            scalar.wait_ge(dsem, 16)
            scalar.activation(
                w_f32[:], pin_s[:, ns : 2 * ns], Act.Exp, scale=-1.0
            ).then_inc(asem, 1)
            # w in bf16 for the band matmuls
            scalar.wait_ge(asem, 1)
            scalar.activation(wb16[:], w_f32[:], Act.Copy).then_inc(wbsem, 1)
            scalar.activation(
                vext[:, WT : WT + 64], pin_s[:, O_SJ : O_SJ + 64], Act.Exp
            ).then_inc(asem, 1)
            scalar.activation(
                vext[:, 0:WT], pin_s[:, O_SB : O_SB + WT], Act.Exp
            ).then_inc(asem, 1)

        @block.gpsimd
        def _(gp):
            gp.iota(idx16[:], [[16, 8]], channel_multiplier=1).then_inc(isem, 1)
            gp.iota(iot[:], [[1, 128 * WMAX]], channel_multiplier=0).then_inc(isem, 1)
            gp.memset(ones_b[:], 1.0).then_inc(isem, 1)
            gp.memset(red[:], 0.0).then_inc(isem, 1)
            if USE_SCATTER_OUT:
                from concourse import library_config
                gp.load_library(library_config.mlp)
                gp.wait_ge(xsem, 1)
                # elem_size 16 (64B packets, min-time descriptors) with
                # elem_step 64 keeping the 256B dst-stride requirement
                gp.dma_scatter_add(
                    out_d[:, 0:16], red[:], idxc[:], 128, 128, 16,
                    elem_step=64, prepare_only=True, sem=osem,
                ).then_inc(psem, 1)
                gp.wait_ge(psem, 1)
                gp.wait_ge(vv, VV_DONE)
                gp.trigger_dma(count=1)

        @block.vector
        def _(vector):
            n = 0

            def step(ins):
                nonlocal n
                n += 1
                ins.then_inc(vv, 1)

            if USE_SCATTER_OUT:
                # clamp the scatter index table: partitions >= 16 are unused
                # by the DMA but must pass its full-tensor bounds check
                vector.wait_ge(isem, 1)
                vector.tensor_scalar(
                    out=idxc[:], in0=idx16[:], scalar1=127.0, scalar2=0.0,
                    op0=Alu.min, op1=Alu.add,
                ).then_inc(xsem, 1)
            vector.wait_ge(dsem, 16)
            vector.wait_ge(isem, 2)
            for q in range(ns):
                step(vector.tensor_scalar(
                    out=mbufs[q][:],
                    in0=iot[:, 0 : 128 * widths[q]],
                    scalar1=pin_s[:, q : q + 1], scalar2=None,
                    op0=Alu.is_ge, op1=Alu.add,
                    accum_out=cntT[:, q : q + 1],
                ))
            assert n == VV_PIECES
            # far-region W broadcast: wcol (psum) -> sbuf -> [ns, 128]
            vector.wait_ge(m1sem, 1)
            step(vector.tensor_copy(wcol_s[:], pwcol[:]))
            vector.wait_ge(vv, ns + 1)
            vector.wait_ge(isem, 3)
            step(vector.tensor_scalar(
                out=wbb[:], in0=ones_b[0:ns, :], scalar1=wcol_s[:, 0:1],
                scalar2=0.0, op0=Alu.mult, op1=Alu.add,
            ))
            # count reductions (fill the gap while PE finishes M2)
            vector.wait_ge(vv, ns)
            vector.wait_ge(isem, 4)
            step(vector.reduce_sum(out=red[:, 0, 2:3], in_=cntT[:], axis=X))
            step(vector.reduce_sum(
                out=red[:, 0, 3:4], in_=pin_s[:, 2 * ns : 3 * ns], axis=X
            ))
            # fused epilogue: sum over (128, WT+64) of (U | WP) * vext
            vector.wait_ge(m2sem, 1)
            vector.wait_ge(asem, 3)
            step(vector.scalar_tensor_tensor(
                out=junke[:], in0=ptile[:], scalar=0.0, in1=vext[:],
                op0=Alu.add, op1=Alu.mult, accum_out=red[:, 0, 0:1],
            ))
            assert n == VV_DONE

        @block.tensor
        def _(tensor):
            # M1: per-slot w column sums  pwcol[q, 0] = sum_p w[p, q]
            tensor.wait_ge(asem, 1)
            tensor.wait_ge(isem, 3)
            tensor.matmul(pwcol[:], w_f32[:], ones_b[:, 0:1],
                          start=True, stop=True).then_inc(m1sem, 1)
            # band matmuls: U[:, off_q + c] += mask_chunk.T @ wb16[:, q]
            tensor.wait_ge(wbsem, 1)
            first = True
            for q in range(ns):
                tensor.wait_ge(vv, q + 1)
                m = mbufs[q]
                for c in range(widths[q]):
                    tensor.matmul(
                        ptile[:, offs[q] + c : offs[q] + c + 1],
                        m[:, 128 * c : 128 * (c + 1)],
                        wb16[:, q : q + 1],
                        start=first, stop=False,
                        skip_group_check=True,
                    )
                    first = False
            # M2: far-region coefficients into the same psum tile
            #   ptile[p, WT + c] = sum_q Wsum_q * hmat[q, c]
            tensor.wait_ge(vv, ns + 2)
            tensor.matmul(ptile[:, WT : WT + 64], wbb[:],
                          pin_s[0:ns, O_H : O_H + 64],
                          start=False, stop=True,
                          skip_group_check=True).then_inc(m2sem, 1)

    if USE_SCATTER_OUT:
        nc.finalize()   # Bacc: run the compile passes (register alloc,
                        # library-load legalization) before PJRT pickup
    return nc


def _plan(preds, targets):
    """Host-side layout prep: sort, band windows, slot dealing."""
    t = np.ascontiguousarray(targets[:, 0], dtype=np.float32)
    e = np.ascontiguousarray(targets[:, 1], dtype=np.float32)
    s = np.ascontiguousarray(preds, dtype=np.float32).reshape(-1)

    orderj = np.argsort(t, kind="stable")
    t_j = t[orderj]
    s_j = s[orderj]

    ev = np.flatnonzero(e != 0.0)
    if len(ev) == 0:
        return None
    ev = ev[np.argsort(t[ev], kind="stable")]
    nblocks = -(-len(ev) // 128)
    nslots = -(-nblocks // NCORES)
    nblocks_pad = nslots * NCORES

    # per-block rows and band boundaries
    bR = np.full((nblocks_pad, 128), 1e9, np.float32)   # searchsorted_right; 1e9 = pad
    bs = np.full((nblocks_pad, 128), 1e30, np.float32)  # s_i (1e30 -> w=0 for pads)
    bn = np.zeros(nblocks_pad, np.int64)                # real event count
    cs = np.zeros(nblocks_pad, np.int64)                # band chunk start (floor)
    ce = np.zeros(nblocks_pad, np.int64)                # band chunk end (ceil)
    for b in range(nblocks):
        idx = ev[b * 128 : (b + 1) * 128]
        k = len(idx)
        pos = np.searchsorted(t_j, t[idx], side="right")
        bR[b, :k] = pos.astype(np.float32)
        bs[b, :k] = s[idx]
        bn[b] = k
        # clamp to the valid chunk range; rows whose pos lands at/after the
        # clamped window end have an empty in-window mask and an empty far
        # region, which is exactly right (no j has t_j > t_i)
        cs[b] = min(int(pos[0]) // 128, NCH - 1)
        ce[b] = min(NCH, max(cs[b] + 1, -(-int(pos[-1]) // 128)))
    bw = ce - cs
    bw[nblocks:] = 0

    # deal blocks into slots by similar band width (desc), one per core
    order_b = np.argsort(-bw, kind="stable")
    widths = []
    slot_blocks = []
    for q in range(nslots):
        grp = order_b[q * NCORES : (q + 1) * NCORES]
        widths.append(max(1, int(bw[grp].max())))
        slot_blocks.append(grp)

    offs = np.concatenate([[0], np.cumsum(widths)]).astype(np.int64)
    WT = int(offs[-1])
    PIN = 3 * nslots + 64 + WT + 64
    O_H = 3 * nslots
    O_SB = 3 * nslots + 64
    O_SJ = 3 * nslots + 64 + WT

    sjb = s_j.reshape(NCH, 128).T
    maps = []
    for c in range(NCORES):
        pin = np.zeros((128, PIN), np.float32)
        pin[:, O_SJ : O_SJ + 64] = sjb
        for q in range(nslots):
            b = slot_blocks[q][c]
            Wq = widths[q]
            # window: Wq chunks ending at ce[b] (extended left, clamped at 0)
            wstart = max(0, int(ce[b]) - Wq) if bn[b] > 0 else 0
            wend = wstart + Wq                     # far region starts here
            # R relative to window start (>= 0 for real rows by construction).
            # Pad sentinel 32000 is exact in both fp32 and int16, in case the
            # ALU evaluates the compare in the int16 domain of the iota.
            pin[:, q] = np.where(
                bR[b] < 1e8,
                np.minimum(bR[b] - np.float32(128 * wstart), np.float32(32000.0)),
                np.float32(32000.0),
            )
            # s' for w = exp(-s'); pads stay 1e30 -> w = 0
            pin[:, nslots + q] = bs[b]
            # far count constants: each real event pairs with all far columns
            nfar = max(0, N - 128 * wend)
            kq = np.zeros(128, np.float32)
            kq[: bn[b]] = np.float32(nfar)
            pin[:, 2 * nslots + q] = kq
            # far-region chunk selector (rows 0..nslots-1 hold hmat)
            if bn[b] > 0 and wend < NCH:
                pin[q, O_H + wend : O_H + NCH] = 1.0
            # s_j at the window's columns
            lo = 128 * wstart
            hi = min(N, 128 * wend)
            pin[:, O_SB + offs[q] : O_SB + offs[q] + (hi - lo) // 128] = (
                s_j[lo:hi].reshape(-1, 128).T
            )
        maps.append({"pin": pin})
    return tuple(widths), maps


def _combine(results):
    loss_sum = 0.0
    count = 0.0
    for r in results:
        part = np.asarray(r["out"], dtype=np.float64)
        loss_sum += part[:, 0].sum() + part[:, 1].sum()
        count += part[:, 2].sum() + part[:, 3].sum()
    return np.array(np.float32(loss_sum) / np.float32(max(count, 1.0)),
                    dtype=np.float32)


def kernel(preds, targets):
    from concourse.bass_utils import run_bass_kernel_spmd

    plan = _plan(preds, targets)
    if plan is None:
        return np.array(0.0, dtype=np.float32)
    widths, maps = plan
    if widths not in _CACHE:
        _CACHE[widths] = _build(widths)
    nc = _CACHE[widths]
    res = run_bass_kernel_spmd(nc, maps, list(range(NCORES)))
    return _combine(res.results)
